# revision 1
# baseline (speedup 1.0000x reference)
"""Causal single-head attention (nn_AttentionHead) on 8 TRN2 NeuronCores, v2.

Same sharding/combine as v1 (8 cores = 4 batches x 2 key-parity shards; host
sums unnormalized flash partials), with a restructured device body:

- x^T is streamed in 4 double-buffered 1024-column chunks (16 KB/partition
  each) instead of held fully resident, so chunk loads of rep n+1 overlap
  compute of rep n and SBUF pressure drops ~4x.
- Projection chunks and attention chunks interleave so attention for query
  chunk qc issues as soon as its kv/q chunks exist.
- Wq is host-duplicated to [Wq|Wq] so the q-projection writes Q^T into both
  PE row groups directly — the per-chunk qt SBUF->SBUF dup DMA disappears.
- The big x loads own the SP queue exclusively; ktd dups issue from DVE and
  output stores from GpSimd, so no small DMA ever stalls the next x load.
- PSUM evacuation of the output accumulator runs on GpSimd, keeping
  Activation free for the 40 exp calls per rep.
"""

import os
import sys
from contextlib import ExitStack

import numpy as np

for _p in ("/root/.axon_site/_ro/trn_rl_repo", "/opt/trn_rl_repo"):
    if os.path.isdir(_p) and _p not in sys.path:
        sys.path.append(_p)

import concourse.bacc as bacc
import concourse.tile as tile
from concourse import mybir

F32 = mybir.dt.float32
F32R = mybir.dt.float32r

B, T, C, H = 4, 4096, 1024, 64
KT = C // 128  # contraction tiles (8)
NKEY = T // 2  # keys per core (2048)
NKT = NKEY // 128  # key tiles per core (16)
NQC = T // 512  # query chunks (8)
NB = T // 128  # 128-row blocks (32)


# ---------------------------------------------------------------- device ----
def build(reps: int = 1):
    nc = bacc.Bacc("TRN2", target_bir_lowering=False, debug=False)

    xq = nc.dram_tensor("xq", [C, T], F32R, kind="ExternalInput")
    wq = nc.dram_tensor("wq", [C, 128], F32R, kind="ExternalInput")  # [Wq|Wq]
    wkv = nc.dram_tensor("wkv", [C, 128], F32R, kind="ExternalInput")  # [Wk|Wv]
    bq8 = nc.dram_tensor("bq8", [128, 1], F32, kind="ExternalInput")  # [bq;bq]/8
    bkv = nc.dram_tensor("bkv", [128, 1], F32, kind="ExternalInput")  # [bk;0]
    ident = nc.dram_tensor("ident", [128, 128], F32R, kind="ExternalInput")
    smask = nc.dram_tensor("smask", [4, 128, 512], F32, kind="ExternalInput")

    outp = nc.dram_tensor("outp", [H + 1, T], F32, kind="ExternalOutput")

    with tile.TileContext(nc) as tc, ExitStack() as ctx:
        const = ctx.enter_context(tc.tile_pool(name="const", bufs=1))
        big = ctx.enter_context(tc.tile_pool(name="big", bufs=1))
        xs = ctx.enter_context(tc.tile_pool(name="xs", bufs=2))
        work = ctx.enter_context(tc.tile_pool(name="work", bufs=3))
        ps = ctx.enter_context(tc.tile_pool(name="ps", bufs=2, space="PSUM"))

        wqt = const.tile([128, KT, 128], F32R)
        nc.sync.dma_start(out=wqt, in_=wq.ap().rearrange("(k p) m -> p k m", p=128))
        wkvt = const.tile([128, KT, 128], F32R)
        nc.sync.dma_start(out=wkvt, in_=wkv.ap().rearrange("(k p) m -> p k m", p=128))
        bq8t = const.tile([128, 1], F32)
        nc.sync.dma_start(out=bq8t, in_=bq8.ap())
        bkvt = const.tile([128, 1], F32)
        nc.sync.dma_start(out=bkvt, in_=bkv.ap())
        idt = const.tile([128, 128], F32R)
        nc.sync.dma_start(out=idt, in_=ident.ap())
        smt = const.tile([128, 4, 512], F32)
        nc.sync.dma_start(out=smt, in_=smask.ap().rearrange("m p t -> p m t"))
        onest = const.tile([128, 1], F32)
        nc.vector.memset(onest, 1.0)

        # persistent per-rep state (bufs=1: same buffer each rep)
        kvt = big.tile([128, NKEY], F32R, tag="kvt")  # 0:64 K^T, 64:128 V^T
        qt = big.tile([128, T], F32R, tag="qt")  # both row groups
        vp = big.tile([128, NKT, H + 1], F32R, tag="vp")
        ktd = big.tile([128, NKEY], F32R, tag="ktd")  # K^T dup at 64:128

        # ones column of vp is constant across reps: fill once
        for t in range(NKT):
            nc.vector.tensor_copy(vp[:, t, H : H + 1], onest)

        for _ in range(reps):
            _body(nc, xs, work, ps, xq, outp, wqt, wkvt, bq8t, bkvt, idt, smt,
                  kvt, qt, vp, ktd)

    nc.compile()
    return nc


def _proj_chunk(nc, ps, xt, xcol0, qc, wqt, wkvt, bq8t, bkvt, kvt, qt, vp, idt, ktd):
    """Project query-chunk qc (512 cols) from streamed x tile xt (first col
    xcol0): kv-proj + V transposes when qc is a key chunk, then q-proj."""
    sl = slice(512 * qc, 512 * (qc + 1))
    lsl = slice(512 * qc - xcol0, 512 * (qc + 1) - xcol0)
    if qc < NQC // 2:
        pkv = ps.tile([128, 512], F32, tag="proj")
        for k in range(KT):
            nc.tensor.matmul(
                pkv, lhsT=wkvt[:, k, :], rhs=xt[:, k, lsl],
                start=(k == 0), stop=(k == KT - 1),
            )
        nc.vector.tensor_scalar_add(kvt[:, sl], pkv, bkvt)
        # K^T dup into row group 64:128 straight from PSUM (no DMA)
        nc.vector.tensor_scalar_add(ktd[64:128, sl], pkv[0:64, :], bkvt[0:64, :])
        for t in range(4 * qc, 4 * qc + 4):
            ptr = ps.tile([128, H], F32R, tag="proj")
            nc.tensor.transpose(
                ptr, kvt[64:128, 128 * t : 128 * (t + 1)], idt[64:128, 64:128]
            )
            nc.vector.tensor_copy(vp[:, t, 0:H], ptr)
    pq = ps.tile([128, 512], F32, tag="proj")
    for k in range(KT):
        nc.tensor.matmul(
            pq, lhsT=wqt[:, k, :], rhs=xt[:, k, lsl],
            start=(k == 0), stop=(k == KT - 1),
        )
    nc.vector.tensor_scalar(
        qt[:, sl], pq, 0.125, bq8t,
        op0=mybir.AluOpType.mult, op1=mybir.AluOpType.add,
    )


def _attn_chunk(nc, work, ps, qc, kvt, qt, vp, smt, ktd, outp):
    """Attention for query chunk qc over its causal key tiles."""
    qsl = slice(512 * qc, 512 * (qc + 1))
    first_half = qc < NQC // 2
    cc = qc if first_half else qc - NQC // 2
    n_k = 4 * cc + 4
    n_p = n_k // 2
    acc = ps.tile([H + 1, 512], F32, tag="acc")

    def scores(jp):
        sp = ps.tile([128, 1024], F32, tag="sp")
        for u in range(2):
            j = 2 * jp + u
            if u == 1:
                lhsT = ktd[64:128, 128 * j : 128 * (j + 1)]
                rhs = qt[64:128, qsl]
            else:
                lhsT = kvt[0:64, 128 * j : 128 * (j + 1)]
                rhs = qt[0:64, qsl]
            nc.tensor.matmul(
                sp[:, 512 * u : 512 * (u + 1)],
                lhsT=lhsT, rhs=rhs,
                start=True, stop=True,
            )
        pt = work.tile([128, 1024], F32R, tag="pt")
        nc.scalar.activation(pt, sp, mybir.ActivationFunctionType.Exp)
        for u in range(2):
            j = 2 * jp + u
            psl = slice(512 * u, 512 * (u + 1))
            d = j - (n_k - 4)
            if d >= 0:
                if first_half:
                    nc.gpsimd.affine_select(
                        out=pt[:, psl], in_=pt[:, psl],
                        pattern=[[1, 512]], channel_multiplier=-1,
                        base=-128 * d, compare_op=mybir.AluOpType.is_ge,
                        fill=0.0,
                    )
                else:
                    nc.vector.tensor_mul(pt[:, psl], pt[:, psl], smt[:, d, :])
        return pt

    def av(jp, pt):
        for u in range(2):
            j = 2 * jp + u
            nc.tensor.matmul(
                acc, lhsT=vp[:, j, :], rhs=pt[:, 512 * u : 512 * (u + 1)],
                start=(j == 0), stop=(j == n_k - 1),
            )

    # software pipeline: scores of pair p+1 issue on PE before AV of pair p,
    # hiding the exp+mask latency from the in-order PE queue
    prev = None
    for jp in range(n_p):
        pt = scores(jp)
        if prev is not None:
            av(jp - 1, prev)
        prev = pt
    av(n_p - 1, prev)
    so = work.tile([H + 1, 512], F32, tag="so")
    nc.vector.tensor_copy(so, acc)
    nc.gpsimd.dma_start(out=outp.ap()[:, qsl], in_=so)


def _body(nc, xs, work, ps, xq, outp, wqt, wkvt, bq8t, bkvt, idt, smt,
          kvt, qt, vp, ktd):
    xq_r = xq.ap().rearrange("(k p) t -> p k t", p=128)
    # chunk i supplies kv chunks {2i, 2i+1} (i<2) and q chunks {2i, 2i+1};
    # attention qc in {2i, 2i+1} (first half) or {2i-4+4, ...} (second half)
    # becomes runnable right after proj of chunk i.
    attn_after = {0: (0, 1), 1: (2, 3), 2: (4, 5), 3: (6, 7)}
    for i in range(4):
        xt = xs.tile([128, KT, 1024], F32R, tag="xt")
        sl = slice(1024 * i, 1024 * (i + 1))
        nc.sync.dma_start(out=xt, in_=xq_r[:, :, sl])
        for qc in (2 * i, 2 * i + 1):
            _proj_chunk(nc, ps, xt, 1024 * i, qc, wqt, wkvt, bq8t, bkvt,
                        kvt, qt, vp, idt, ktd)
        for qc in attn_after[i]:
            _attn_chunk(nc, work, ps, qc, kvt, qt, vp, smt, ktd, outp)


# ------------------------------------------------------------------ host ----
def _perm_cols(h):
    blocks = list(range(h, NB, 2)) + list(range(1 - h, NB, 2))
    return np.concatenate([np.arange(128 * g, 128 * (g + 1)) for g in blocks])


def _step_masks(h):
    m = np.zeros((4, 128, 512), dtype=np.float32)
    for t in range(4):
        for i in range(4):
            if (i >= t) if h == 0 else (i >= t + 1):
                m[t, :, 128 * i : 128 * (i + 1)] = 1.0
    return m


def _make_in_maps(batch_x, Wk, bk, Wq, bq, Wv):
    xT = np.ascontiguousarray(np.transpose(batch_x, (0, 2, 1)))
    wkv = np.ascontiguousarray(
        np.concatenate([Wk, Wv], axis=1).astype(np.float32)
    )
    wq_c = np.asarray(Wq, dtype=np.float32)
    wq2 = np.ascontiguousarray(np.concatenate([wq_c, wq_c], axis=1))
    bq8_h = (np.asarray(bq, dtype=np.float32) * 0.125).reshape(H)
    bq8 = np.concatenate([bq8_h, bq8_h]).reshape(128, 1)
    bkv = np.concatenate(
        [np.asarray(bk, dtype=np.float32), np.zeros(64, np.float32)]
    ).reshape(128, 1)
    ident = np.eye(128, dtype=np.float32)
    cols = {h: _perm_cols(h) for h in (0, 1)}
    masks = {h: _step_masks(h) for h in (0, 1)}
    return [
        {
            "xq": np.ascontiguousarray(xT[b][:, cols[h]]),
            "wq": wq2,
            "wkv": wkv,
            "bq8": bq8,
            "bkv": bkv,
            "ident": ident,
            "smask": masks[h],
        }
        for b in range(B)
        for h in (0, 1)
    ]


def _combine(outps, bv):
    inv = {}
    for h in (0, 1):
        c = _perm_cols(h)
        inv[h] = np.empty_like(c)
        inv[h][c] = np.arange(T)
    out = np.empty((B, T, H), dtype=np.float32)
    for b in range(B):
        tot = np.zeros((H + 1, T), dtype=np.float64)
        for h in (0, 1):
            o = np.asarray(outps[2 * b + h], dtype=np.float64)
            tot += o[:, inv[h]]
        out[b] = (tot[0:H] / tot[H]).T + bv.astype(np.float64)
    return out


_CACHE = {}


def _get_nc():
    if "nc" not in _CACHE:
        _CACHE["nc"] = build(reps=1)
    return _CACHE["nc"]


def kernel(batch_x, Wk, bk, Wq, bq, Wv, bv):
    from concourse.bass_utils import run_bass_kernel_spmd

    batch_x = np.asarray(batch_x, dtype=np.float32)
    in_maps = _make_in_maps(
        batch_x, np.asarray(Wk), np.asarray(bk), np.asarray(Wq),
        np.asarray(bq), np.asarray(Wv),
    )
    nc = _get_nc()
    res = run_bass_kernel_spmd(nc, in_maps, core_ids=list(range(8)))
    outps = [res.results[c]["outp"] for c in range(8)]
    return _combine(outps, np.asarray(bv))

